# revision 13
# baseline (speedup 1.0000x reference)
"""Trainium2 Bass kernel for a Llama block (B=2, S=2048, D=2048, H=16, FF=8192).

Sharding (8 cores, fully static SPMD — one program, per-core differences are
input data only):
  - core c (batch b = c//4, p = c%4) owns query chunks {p, p+4, p+8, p+12}
    (each 128 tokens) of its batch: Q projection, attention queries, WO,
    norm2 and the FFN for those 512 tokens.
  - core c computes K/V projections for own heads [4p, 4p+4) of its batch;
    four group-of-4 AllGathers (one per own head j, K+V together) pipeline
    against Q projection and the attention rounds: after AG_j the core holds
    heads {j, 4+j, 8+j, 12+j}, processed as attention round j.
  - attention is causal with uniform suffix widths: at key block kb only
    query slots i >= kb//4 are live; the first live slot gets an additive
    mask selected by kb%4 from a per-core [128, 4, 128] table (zeros /
    triangular / -inf), so the instruction stream is core-invariant.
  - normalized activations nx = rmsnorm(x) of the whole batch are computed
    once into SBUF (bf16, 64 KB/partition) and reused by K and V.
  - FFN runs in fp8 (e4m3, weights host-scaled x16) with DoubleRow matmuls.
  - activations live in [feature, token] layout throughout; the host passes x
    pre-transposed and reassembles the transposed per-core outputs.
  - RMSNorm weights are folded into the following projection weights on the
    host (exact here since they are 1.0, and algebraically valid in general).
"""

import contextlib
import math
import os
import sys
import types

sys.path.insert(0, "/opt/trn_rl_repo")


def _install_ntff_hook_shim():
    """The image's antenv package lacks axon_hooks; provide it so
    run_bass_kernel_spmd(trace=True) can fetch the NTFF profile hook.
    Degrades silently if the boot .so lacks the profiling symbols."""
    if "antenv.axon_hooks" in sys.modules:
        return
    mod = types.ModuleType("antenv.axon_hooks")
    mod._HOOK = None
    mod.set_axon_ntff_profile_hook = lambda h: setattr(mod, "_HOOK", h)
    mod.get_axon_ntff_profile_hook = lambda: mod._HOOK
    sys.modules["antenv.axon_hooks"] = mod
    try:
        if "/root/.axon_site" not in sys.path:
            sys.path.append("/root/.axon_site")
        from trn_agent_boot.trn_boot import _ntff_profile_via_ctypes

        mod._HOOK = _ntff_profile_via_ctypes("/opt/axon/libaxon_pjrt.so")
    except Exception:
        pass


_install_ntff_hook_shim()

import ml_dtypes
import numpy as np

import concourse.bass as bass
import concourse.mybir as mybir
import concourse.tile as tile
from concourse import bacc
from concourse.bass_utils import run_bass_kernel_spmd

F32 = mybir.dt.float32
F32R = mybir.dt.float32r
BF16 = mybir.dt.bfloat16
F8 = mybir.dt.float8e4
AFT = mybir.ActivationFunctionType
ALU = mybir.AluOpType
DR = mybir.MatmulPerfMode.DoubleRow
W8SCALE = 16.0          # fp8 FFN weights are host-scaled by this

B, S, D, H = 2, 2048, 2048, 16
HD = D // H            # 128
FF = 4 * D             # 8192
NC = 8
TOK = 512              # own tokens per core
EPS = 1e-6
BASE = 10000.0
NEG = -1e30
P = 128
DCH = D // P           # 16 d-chunks
FCH = FF // P          # 64 ff subchunks
SCALE = 1.0 / math.sqrt(HD)
KB = S // P            # 16 key blocks
HDS = HD * S           # one head's K (or V) element count

_CACHE = {}
LAST_RESULT = None


def _rope_tables(positions):
    """[128, n] cos/sin tables with the 64-row table duplicated in both
    partition halves (for lane-aligned rope on-device)."""
    t = BASE ** (-2.0 * (np.arange(HD // 2, dtype=np.float64) - 1.0) / HD)
    ang = positions[:, None].astype(np.float64) * t[None, :]       # [n, 64]
    c = np.cos(ang).T.astype(np.float32)
    sn = np.sin(ang).T.astype(np.float32)
    return (np.concatenate([c, c], axis=0), np.concatenate([sn, sn], axis=0))


def _build_program():
    nc = bacc.Bacc("TRN2", target_bir_lowering=False, debug=False,
                   num_devices=NC)

    def inp(name, shape, dtype=F32R):
        return nc.dram_tensor(name, shape, dtype, kind="ExternalInput").ap()

    xT_b = inp("xT_b", [D, S])             # own batch, transposed
    xT_own = inp("xT_own", [D, TOK])       # own query chunks, transposed
    wq = inp("wq", [H, P, DCH, HD], BF16)      # pre-tiled [h, p, o, f]
    wk = inp("wk", [D, 4 * HD], BF16)
    wv = inp("wv", [D, 4 * HD], BF16)
    wo = inp("wo", [DCH, P, DCH, P], BF16)     # pre-tiled [o, p, a, f]
    wg = inp("wg", [FCH, P, DCH, P], F8)       # pre-tiled [fb, p, o, f], x16
    wu = inp("wu", [FCH, P, DCH, P], F8)       # pre-tiled [fb, p, o, f], x16
    wd = inp("wd", [4, DCH, P, DCH, P], F8)    # pre-tiled [sc, o, p, fs, f], x16
    bq = inp("bq", [P, H], F32)
    bk = inp("bk", [P, 4], F32)
    bvb = inp("bvb", [P, 4 * HD], F32)
    bo = inp("bo", [P, DCH], F32)
    bg = inp("bg", [P, FCH], F32)
    bu = inp("bu", [P, FCH], F32)
    bd = inp("bd", [P, DCH], F32)
    cosk = inp("cosk", [P, S], F32)
    sink = inp("sink", [P, S], F32)
    cosq = inp("cosq", [P, TOK], F32)
    sinq = inp("sinq", [P, TOK], F32)
    mask4 = inp("mask4", [P, 4, P], F32)   # per-core additive causal masks
    onesb = inp("onesb", [P, 1], BF16)
    epsv = inp("epsv", [P, 1], F32)
    out_t = nc.dram_tensor("out", [D, TOK], F32, kind="ExternalOutput").ap()

    xT_b3 = xT_b.rearrange("(o p) t -> p o t", p=P)
    xT_own3 = xT_own.rearrange("(o p) t -> p o t", p=P)

    with tile.TileContext(nc) as tc:
        with tc.tile_pool(name="consts", bufs=1) as consts, \
             tc.tile_pool(name="big", bufs=1) as big, \
             tc.tile_pool(name="dram", bufs=1, space="DRAM") as dram:
            onesb_s = consts.tile([P, 1], BF16)
            nc.sync.dma_start(onesb_s[:], onesb[:])
            eps_s = consts.tile([P, 1], F32)
            nc.sync.dma_start(eps_s[:], epsv[:])
            bq_s = consts.tile([P, H], F32)
            nc.sync.dma_start(bq_s[:], bq[:])
            bk_s = consts.tile([P, 4], F32)
            nc.sync.dma_start(bk_s[:], bk[:])
            bvb_s = consts.tile([P, 4 * HD], F32)
            nc.sync.dma_start(bvb_s[:], bvb[:])
            bo_s = consts.tile([P, DCH], F32)
            nc.sync.dma_start(bo_s[:], bo[:])
            bg_s = consts.tile([P, FCH], F32)
            nc.sync.dma_start(bg_s[:], bg[:])
            bu_s = consts.tile([P, FCH], F32)
            nc.sync.dma_start(bu_s[:], bu[:])
            bd_s = consts.tile([P, DCH], F32)
            nc.sync.dma_start(bd_s[:], bd[:])
            mask_s = consts.tile([P, 4, P], F32)
            nc.sync.dma_start(mask_s[:], mask4[:])
            cosq_s = consts.tile([P, TOK], F32)
            nc.sync.dma_start(cosq_s[:], cosq[:])
            sinq_s = consts.tile([P, TOK], F32)
            nc.sync.dma_start(sinq_s[:], sinq[:])

            # per-own-head K+V bounce (1 MB) and gather (4 MB) buffers
            kv_bounce = [dram.tile([2 * HDS], BF16, name=f"kvb{j}")
                         for j in range(4)]
            kv_gath = [dram.tile([4 * 2 * HDS], BF16, name=f"kvg{j}")
                       for j in range(4)]
            groups = [[0, 1, 2, 3], [4, 5, 6, 7]]

            # big SBUF slots (aliased across phases by tag)
            #   A (64K): nx_all (P1-P2) -> attention round K/V double-buffer
            #   B (32K): kv_ws (P2) -> acc (P5-P7)
            #   C (16K): cskt rope tables (P2) -> att_all (P4-P5) -> act2 (P6)
            #   D (16K): qt_all (P3-P4) -> nx2 (P6)
            nx_all = big.tile([P, DCH, S], BF16, tag="bigA", name="nx_all")
            kv_ws = big.tile([P, 2, DCH, 4 * HD], BF16, tag="bigB",
                             name="kv_ws")
            cskt = big.tile([P, 2, S], F32, tag="bigC", name="cskt")
            nc.sync.dma_start(cskt[:, 0, :], cosk[:])
            nc.sync.dma_start(cskt[:, 1, :], sink[:])
            nc.sync.dma_start(kv_ws[:, 0],
                              wk.rearrange("(o p) f -> p o f", p=P))
            nc.sync.dma_start(kv_ws[:, 1],
                              wv.rearrange("(o p) f -> p o f", p=P))

            def rope(pool, src, cos_t, sin_t, dst, tname):
                """src [128, n] f32 pre-rope -> dst [128, n] roped.

                cos_t/sin_t are [128, n] with the 64-row table duplicated in
                both partition halves. A half-swapped copy of src keeps every
                elementwise op lane-aligned:
                  ma = src*cos  -> [f*cos ; s*cos]
                  mb = swap(src)*sin -> [s*sin ; f*sin]
                  dst = [ma_top + mb_top ; mb_bot - ma_bot]
                """
                n = src.shape[-1]
                hh = HD // 2
                swp = pool.tile([P, n], F32, tag="rpsw", name=f"{tname}sw")
                nc.sync.dma_start(swp[0:hh, :], src[hh:P, :])
                nc.sync.dma_start(swp[hh:P, :], src[0:hh, :])
                ma = pool.tile([P, n], F32, tag="rp1", name=f"{tname}ma")
                mb = pool.tile([P, n], F32, tag="rp2", name=f"{tname}mb")
                nc.vector.tensor_mul(out=ma[:], in0=src[:], in1=cos_t)
                nc.vector.tensor_mul(out=mb[:], in0=swp[:], in1=sin_t)
                nc.vector.tensor_add(out=dst[0:hh], in0=ma[0:hh],
                                     in1=mb[0:hh])
                nc.vector.tensor_sub(out=dst[hh:P], in0=mb[hh:P],
                                     in1=ma[hh:P])

            # ---- P1: load x once, rmsnorm into SBUF (nx_all, bf16) ----
            with tc.tile_pool(name="p1", bufs=3) as pool, \
                 tc.tile_pool(name="p1ps", bufs=2, space="PSUM") as psum:
                for tb in range(S // TOK):
                    cols = bass.ts(tb, TOK)
                    sumsq = psum.tile([1, TOK], F32, tag="n1ss",
                                      name=f"n1ss{tb}")
                    for o in range(DCH):
                        xc = pool.tile([P, TOK], F32R, tag="n1x",
                                       name=f"n1x{tb}_{o}")
                        nc.sync.dma_start(xc[:], xT_b3[:, o, cols])
                        dst = nx_all[:, o, cols]
                        eng = (nc.scalar, nc.vector, nc.gpsimd)[o % 3]
                        if eng is nc.scalar:
                            nc.scalar.copy(dst, xc[:])
                        else:
                            eng.tensor_scalar_add(dst, xc[:].bitcast(F32), 0.0)
                        sq = pool.tile([P, TOK], BF16, tag="n1sq",
                                       name=f"n1sq{tb}_{o}")
                        (nc.vector if o % 2 else nc.gpsimd).tensor_mul(
                            out=sq[:], in0=dst, in1=dst)
                        nc.tensor.matmul(sumsq[:], lhsT=onesb_s[:], rhs=sq[:],
                                         start=(o == 0), stop=(o == DCH - 1))
                    rms = pool.tile([1, TOK], F32, tag="n1rms",
                                    name=f"n1rms{tb}")
                    nc.scalar.activation(rms[:], sumsq[:], AFT.Sqrt,
                                         scale=1.0 / D, bias=eps_s[:1])
                    rec = pool.tile([1, TOK], F32, tag="n1rec",
                                    name=f"n1rec{tb}")
                    nc.vector.reciprocal(rec[:], rms[:])
                    rbc = pool.tile([P, TOK], F32, tag="n1rbc",
                                    name=f"n1rbc{tb}")
                    nc.gpsimd.partition_broadcast(rbc[:], rec[:])
                    for o in range(DCH):
                        dst = nx_all[:, o, cols]
                        eng = (nc.vector, nc.gpsimd)[o % 2]
                        eng.tensor_mul(out=dst, in0=dst, in1=rbc[:])

            # ---- P2a: V projections for own 4 heads (wide rhs) ----
            with tc.tile_pool(name="p2a", bufs=2) as pool, \
                 tc.tile_pool(name="p2aps", bufs=2, space="PSUM") as psum:
                for tb in range(S // TOK):
                    vps = [psum.tile([P, 4 * HD], F32, tag=f"vps{i}",
                                     name=f"vps{i}_{tb}") for i in range(4)]
                    for o in range(DCH):
                        st, sp = (o == 0), (o == DCH - 1)
                        for ts_ in range(4):
                            nc.tensor.matmul(
                                vps[ts_][:],
                                lhsT=nx_all[:, o,
                                            bass.ds(tb * TOK + ts_ * P, P)],
                                rhs=kv_ws[:, 1, o, :], start=st, stop=sp)
                    for ts_ in range(4):
                        vsb = pool.tile([P, 4 * HD], BF16, tag="vsb",
                                        name=f"vsb{tb}_{ts_}")
                        nc.vector.tensor_add(out=vsb[:], in0=vps[ts_][:],
                                             in1=bvb_s[:])
                        row0 = tb * TOK + ts_ * P
                        for j in range(4):
                            vdst = kv_bounce[j][HDS:2 * HDS].rearrange(
                                "(s hd) -> s hd", hd=HD)
                            nc.sync.dma_start(
                                vdst[bass.ds(row0, P), :],
                                vsb[:, bass.ts(j, HD)])

            # ---- P2b: K proj per own head j + rope -> AG_j (K+V) ----
            with tc.tile_pool(name="p2b", bufs=2) as pool, \
                 tc.tile_pool(name="p2bps", bufs=2, space="PSUM") as psum:
                for j in range(4):
                    kdst = kv_bounce[j][0:HDS].rearrange("(hd s) -> hd s", s=S)
                    for tb in range(S // TOK):
                        cols = bass.ts(tb, TOK)
                        kps = psum.tile([P, TOK], F32, tag="kps",
                                        name=f"kps{j}_{tb}")
                        for o in range(DCH):
                            nc.tensor.matmul(
                                kps[:], lhsT=kv_ws[:, 0, o, bass.ts(j, HD)],
                                rhs=nx_all[:, o, cols],
                                start=(o == 0), stop=(o == DCH - 1))
                        kb_t = pool.tile([P, TOK], F32, tag="kbias",
                                         name=f"kbias{j}_{tb}")
                        nc.scalar.activation(kb_t[:], kps[:], AFT.Identity,
                                             bias=bk_s[:, j:j + 1])
                        krt = pool.tile([P, TOK], BF16, tag="kroped",
                                        name=f"kroped{j}_{tb}")
                        rope(pool, kb_t[:], cskt[:, 0, cols], cskt[:, 1, cols],
                             krt[:], f"kr{j}_{tb}")
                        nc.sync.dma_start(kdst[:, cols], krt[:])
                    nc.gpsimd.collective_compute(
                        "AllGather", mybir.AluOpType.bypass,
                        ins=[kv_bounce[j][:].opt()],
                        outs=[kv_gath[j][:].opt()],
                        replica_groups=groups)

            # ---- P3: Q projections for own tokens, all heads + rope ----
            qt_all = big.tile([P, H, TOK], BF16, tag="bigD", name="qt_all")
            with tc.tile_pool(name="p3", bufs=2) as pool, \
                 tc.tile_pool(name="p3q", bufs=1) as poolq, \
                 tc.tile_pool(name="p3ps", bufs=2, space="PSUM") as psum:
                nxq = poolq.tile([P, DCH, TOK], BF16, tag="nxq")
                rbcq = poolq.tile([P, TOK], F32, tag="rbcq")
                sumsq = psum.tile([1, TOK], F32, tag="qss")
                for o in range(DCH):
                    xc = pool.tile([P, TOK], F32R, tag="qx", name=f"qx{o}")
                    nc.sync.dma_start(xc[:], xT_own3[:, o, :])
                    dst = nxq[:, o, :]
                    eng = (nc.scalar, nc.vector, nc.gpsimd)[o % 3]
                    if eng is nc.scalar:
                        nc.scalar.copy(dst, xc[:])
                    else:
                        eng.tensor_scalar_add(dst, xc[:].bitcast(F32), 0.0)
                    sq = pool.tile([P, TOK], BF16, tag="qsq", name=f"qsq{o}")
                    (nc.vector if o % 2 else nc.gpsimd).tensor_mul(
                        out=sq[:], in0=dst, in1=dst)
                    nc.tensor.matmul(sumsq[:], lhsT=onesb_s[:], rhs=sq[:],
                                     start=(o == 0), stop=(o == DCH - 1))
                rms = pool.tile([1, TOK], F32, tag="qrms")
                nc.scalar.activation(rms[:], sumsq[:], AFT.Sqrt,
                                     scale=1.0 / D, bias=eps_s[:1])
                rec = pool.tile([1, TOK], F32, tag="qrec")
                nc.vector.reciprocal(rec[:], rms[:])
                nc.gpsimd.partition_broadcast(rbcq[:], rec[:])
                for o in range(DCH):
                    eng = (nc.vector, nc.gpsimd)[o % 2]
                    eng.tensor_mul(out=nxq[:, o, :], in0=nxq[:, o, :],
                                   in1=rbcq[:])
                for h in range(H):
                    wq_s = pool.tile([P, DCH, HD], BF16, tag="wqs",
                                     name=f"wqs{h}")
                    nc.sync.dma_start(wq_s[:], wq[h])
                    qp = psum.tile([P, TOK], F32, tag="qps", name=f"qps{h}")
                    for o in range(DCH):
                        nc.tensor.matmul(qp[:], lhsT=wq_s[:, o, :],
                                         rhs=nxq[:, o, :],
                                         start=(o == 0),
                                         stop=(o == DCH - 1))
                    qb_t = pool.tile([P, TOK], F32, tag="qbias",
                                     name=f"qbias{h}")
                    nc.scalar.activation(qb_t[:], qp[:], AFT.Identity,
                                         bias=bq_s[:, h:h + 1])
                    rope(pool, qb_t[:], cosq_s[:], sinq_s[:],
                         qt_all[:, h, :], f"qr{h}")

            # ---- P4: attention rounds (causal, stride-4 query chunks) ----
            # round j processes heads {4m+j}; K/V from AG_j.  At key block kb
            # only query slots i >= kb//4 are live (suffix); the first live
            # slot gets the additive mask mask_s[:, kb%4, :].
            att_all = big.tile([P, H, TOK], BF16, tag="bigC", name="att_all")
            kvr = big.tile([P, 2, 4, 2, S], BF16, tag="bigA", name="kvr")
            with tc.tile_pool(name="p4", bufs=3) as pool, \
                 tc.tile_pool(name="p4ps", bufs=2, space="PSUM") as psum:
                for j in range(4):
                    r2 = j % 2
                    for m in range(4):
                        h = 4 * m + j
                        ktg = kvr[:, r2, m, 0, :]
                        nc.sync.dma_start(
                            ktg,
                            kv_gath[j][bass.ds(m * 2 * HDS, HDS)].rearrange(
                                "(hd s) -> hd s", s=S))
                        vg = kvr[:, r2, m, 1, :].rearrange(
                            "p (kb hd) -> p kb hd", hd=HD)
                        nc.sync.dma_start(
                            vg,
                            kv_gath[j][bass.ds(m * 2 * HDS + HDS,
                                               HDS)].rearrange(
                                "(kb p hd) -> p kb hd", p=P, hd=HD))
                        den = psum.tile([1, TOK], F32, tag="denps",
                                        name=f"den{h}")
                        op = psum.tile([P, TOK], F32, tag="outps",
                                       name=f"ops{h}")
                        for kb in range(KB):
                            off = (kb // 4) * P
                            stp = psum.tile([P, TOK], F32, tag="stps",
                                            name=f"st{h}_{kb}")
                            nc.tensor.matmul(stp[:, off:TOK],
                                             lhsT=ktg[:, bass.ts(kb, P)],
                                             rhs=qt_all[:, h, off:TOK],
                                             start=True, stop=True)
                            nc.vector.tensor_add(out=stp[:, off:off + P],
                                                 in0=stp[:, off:off + P],
                                                 in1=mask_s[:, kb % 4, :])
                            est = pool.tile([P, TOK], BF16, tag="est",
                                            name=f"est{h}_{kb}")
                            nc.scalar.activation(est[:, off:TOK],
                                                 stp[:, off:TOK],
                                                 AFT.Exp, scale=SCALE)
                            st, sp = (kb == 0), (kb == KB - 1)
                            nc.tensor.matmul(den[:, off:TOK], lhsT=onesb_s[:],
                                             rhs=est[:, off:TOK],
                                             start=st, stop=sp,
                                             skip_group_check=True)
                            nc.tensor.matmul(op[:, off:TOK],
                                             lhsT=vg[:, kb, :],
                                             rhs=est[:, off:TOK],
                                             start=st, stop=sp,
                                             skip_group_check=True)
                        recd = pool.tile([1, TOK], F32, tag="recd",
                                         name=f"recd{h}")
                        nc.vector.reciprocal(recd[:], den[:])
                        rdb = pool.tile([P, TOK], F32, tag="rdb",
                                        name=f"rdb{h}")
                        nc.gpsimd.partition_broadcast(rdb[:], recd[:])
                        nc.vector.tensor_mul(out=att_all[:, h, :], in0=op[:],
                                             in1=rdb[:])

            # ---- P5: WO for own tokens + residual -> acc (= x2T) ----
            acc = big.tile([P, DCH, TOK], F32, tag="bigB", name="acc")
            with tc.tile_pool(name="p5", bufs=2) as pool, \
                 tc.tile_pool(name="p5ps", bufs=2, space="PSUM") as psum:
                for o in range(DCH):
                    wo_s = pool.tile([P, DCH, P], BF16, tag="wos",
                                     name=f"wos{o}")
                    nc.sync.dma_start(wo_s[:], wo[o])
                    x2p = psum.tile([P, TOK], F32, tag="x2ps", name=f"x2ps{o}")
                    for h in range(H):
                        nc.tensor.matmul(x2p[:], lhsT=wo_s[:, h, :],
                                         rhs=att_all[:, h, :],
                                         start=(h == 0), stop=(h == H - 1))
                    x2pre = pool.tile([P, TOK], F32, tag="x2pre",
                                      name=f"x2pre{o}")
                    nc.scalar.activation(x2pre[:], x2p[:], AFT.Identity,
                                         bias=bo_s[:, o:o + 1])
                    xres = pool.tile([P, TOK], F32R, tag="xres",
                                     name=f"xres{o}")
                    nc.sync.dma_start(xres[:], xT_own3[:, o, :])
                    nc.vector.tensor_add(out=acc[:, o, :], in0=x2pre[:],
                                         in1=xres[:].bitcast(F32))

            # ---- P6: norm2 + FFN (fp8 weights/activations, DoubleRow) ----
            nx2 = big.tile([P, DCH, TOK], F8, tag="bigD", name="nx2")
            act2 = big.tile([P, 2, DCH, TOK], F8, tag="bigC", name="act2")
            with tc.tile_pool(name="p6w", bufs=3) as wpool6, \
                 tc.tile_pool(name="p6", bufs=2) as pool, \
                 tc.tile_pool(name="p6ps", bufs=2, space="PSUM") as psum:
                rbc2 = pool.tile([P, TOK], F32, tag="rbc2")
                sumsq = psum.tile([1, TOK], F32, tag="n2ss")
                for o in range(DCH):
                    sq = pool.tile([P, TOK], BF16, tag="n2sq", name=f"n2sq{o}")
                    nc.scalar.activation(sq[:], acc[:, o, :], AFT.Square)
                    nc.tensor.matmul(sumsq[:], lhsT=onesb_s[:], rhs=sq[:],
                                     start=(o == 0), stop=(o == DCH - 1))
                rms = pool.tile([1, TOK], F32, tag="n2rms")
                nc.scalar.activation(rms[:], sumsq[:], AFT.Sqrt,
                                     scale=1.0 / D, bias=eps_s[:1])
                rec = pool.tile([1, TOK], F32, tag="n2rec")
                nc.vector.reciprocal(rec[:], rms[:])
                nc.gpsimd.partition_broadcast(rbc2[:], rec[:])
                for o in range(DCH):
                    nc.vector.tensor_mul(out=nx2[:, o, :], in0=acc[:, o, :],
                                         in1=rbc2[:])
                # fold b_down into acc now (added once)
                for o in range(DCH):
                    nc.vector.tensor_scalar_add(acc[:, o, :], acc[:, o, :],
                                                bd_s[:, o:o + 1])
                for sc in range(4):
                    for fs in range(DCH):
                        f = sc * DCH + fs
                        wg_s = wpool6.tile([P, DCH, P], F8, tag="wgs",
                                           name=f"wgs{f}")
                        nc.sync.dma_start(wg_s[:], wg[f])
                        wu_s = wpool6.tile([P, DCH, P], F8, tag="wus",
                                           name=f"wus{f}")
                        nc.sync.dma_start(wu_s[:], wu[f])
                        gp = psum.tile([P, TOK], F32, tag="gps", name=f"gps{f}")
                        up = psum.tile([P, TOK], F32, tag="ups", name=f"ups{f}")
                        for oj in range(DCH // 2):
                            st, sp = (oj == 0), (oj == DCH // 2 - 1)
                            o2 = bass.ds(2 * oj, 2)
                            nc.tensor.matmul(gp[:], lhsT=wg_s[:, o2, :],
                                             rhs=nx2[:, o2, :], start=st,
                                             stop=sp, perf_mode=DR)
                            nc.tensor.matmul(up[:], lhsT=wu_s[:, o2, :],
                                             rhs=nx2[:, o2, :], start=st,
                                             stop=sp, perf_mode=DR)
                        gs = pool.tile([P, TOK], F32, tag="gsig", name=f"gs{f}")
                        nc.scalar.activation(gs[:], gp[:], AFT.Silu,
                                             scale=1.0 / W8SCALE,
                                             bias=bg_s[:, f:f + 1])
                        us = pool.tile([P, TOK], F32, tag="usig", name=f"us{f}")
                        nc.scalar.activation(us[:], up[:], AFT.Identity,
                                             scale=1.0 / W8SCALE,
                                             bias=bu_s[:, f:f + 1])
                        nc.vector.tensor_mul(out=act2[:, sc % 2, fs, :],
                                             in0=gs[:], in1=us[:])
                    for o in range(DCH):
                        wd_s = wpool6.tile([P, DCH, P], F8, tag="wds",
                                           name=f"wds{sc}_{o}")
                        nc.sync.dma_start(wd_s[:], wd[sc, o])
                        dp = psum.tile([P, TOK], F32, tag="dps",
                                       name=f"dps{sc}_{o}")
                        for fj in range(DCH // 2):
                            f2 = bass.ds(2 * fj, 2)
                            nc.tensor.matmul(dp[:], lhsT=wd_s[:, f2, :],
                                             rhs=act2[:, sc % 2, f2, :],
                                             start=(fj == 0),
                                             stop=(fj == DCH // 2 - 1),
                                             perf_mode=DR)
                        nc.vector.scalar_tensor_tensor(
                            out=acc[:, o, :], in0=dp[:],
                            scalar=1.0 / W8SCALE, in1=acc[:, o, :],
                            op0=ALU.mult, op1=ALU.add)

            # ---- P7: write transposed output ----
            nc.sync.dma_start(
                out_t.rearrange("(o p) t -> p o t", p=P), acc[:])

    nc.compile()
    return nc


def _prepare_inputs(inputs):
    """Build the 8 per-core in_maps from the full problem inputs."""
    x = np.ascontiguousarray(inputs["x"], dtype=np.float32)   # [B, S, D]
    n1 = np.asarray(inputs["norm1_w"], dtype=np.float32)
    n2 = np.asarray(inputs["norm2_w"], dtype=np.float32)
    wq_f = np.ascontiguousarray(n1[:, None] * np.asarray(inputs["wq"], np.float32))
    wk_f = n1[:, None] * np.asarray(inputs["wk"], np.float32)
    wv_f = n1[:, None] * np.asarray(inputs["wv"], np.float32)
    wo_f = np.ascontiguousarray(np.asarray(inputs["wo"], np.float32))
    wg_f = np.ascontiguousarray(n2[:, None] * np.asarray(inputs["w_gate"], np.float32))
    wu_f = np.ascontiguousarray(n2[:, None] * np.asarray(inputs["w_up"], np.float32))
    wd_f = np.ascontiguousarray(np.asarray(inputs["w_down"], np.float32))
    bq = np.asarray(inputs["bq"], np.float32).reshape(H, P).T.copy()
    bo = np.asarray(inputs["bo"], np.float32).reshape(DCH, P).T.copy()
    bg = np.asarray(inputs["b_gate"], np.float32).reshape(FCH, P).T.copy()
    bu = np.asarray(inputs["b_up"], np.float32).reshape(FCH, P).T.copy()
    bd = np.asarray(inputs["b_down"], np.float32).reshape(DCH, P).T.copy()
    bk_full = np.asarray(inputs["bk"], np.float32)
    bv_full = np.asarray(inputs["bv"], np.float32)

    cosk, sink = _rope_tables(np.arange(S))
    onesb_np = np.ones((P, 1), ml_dtypes.bfloat16)
    epsv = np.full((P, 1), EPS, np.float32)

    xT = [np.ascontiguousarray(x[b].T) for b in range(B)]      # [D, S]
    bf = ml_dtypes.bfloat16
    # pre-tiled layouts so every weight-tile DMA is one contiguous block
    wq_b = np.ascontiguousarray(
        wq_f.astype(bf).reshape(DCH, P, H, HD).transpose(2, 1, 0, 3))
    wk_b = wk_f.astype(bf)
    wv_b = wv_f.astype(bf)
    wo_b = np.ascontiguousarray(
        wo_f.astype(bf).reshape(DCH, P, DCH, P).transpose(2, 1, 0, 3))
    f8 = ml_dtypes.float8_e4m3
    wg_b = np.ascontiguousarray(
        (W8SCALE * wg_f).astype(f8).reshape(DCH, P, FCH, P).transpose(2, 1, 0, 3))
    wu_b = np.ascontiguousarray(
        (W8SCALE * wu_f).astype(f8).reshape(DCH, P, FCH, P).transpose(2, 1, 0, 3))
    wd_b = np.ascontiguousarray(
        (W8SCALE * wd_f).astype(f8).reshape(4, DCH, P, DCH, P).transpose(0, 3, 2, 1, 4))

    tri = np.where(np.arange(P)[:, None] > np.arange(P)[None, :],
                   NEG, 0.0).astype(np.float32)
    in_maps = []
    for c in range(NC):
        b, p = c // 4, c % 4
        kv0 = 4 * p * HD                                        # head-col base
        # owned query chunks: slots i = 0..3 hold chunk p + 4i (128 tokens)
        qpos = np.concatenate([np.arange(P) + (p + 4 * i) * P
                               for i in range(4)])
        cosq, sinq = _rope_tables(qpos)
        mask4 = np.empty((P, 4, P), np.float32)
        for j in range(4):
            if j < p:
                mask4[:, j, :] = 0.0
            elif j == p:
                mask4[:, j, :] = tri
            else:
                mask4[:, j, :] = NEG
        in_maps.append({
            "xT_b": xT[b],
            "xT_own": np.ascontiguousarray(xT[b][:, qpos]),
            "wq": wq_b,
            "wk": np.ascontiguousarray(wk_b[:, kv0:kv0 + 4 * HD]),
            "wv": np.ascontiguousarray(wv_b[:, kv0:kv0 + 4 * HD]),
            "wo": wo_b,
            "wg": wg_b, "wu": wu_b, "wd": wd_b,
            "bq": bq,
            "bk": bk_full[kv0:kv0 + 4 * HD].reshape(4, P).T.copy(),
            "bvb": np.tile(bv_full[kv0:kv0 + 4 * HD][None, :], (P, 1)).copy(),
            "bo": bo, "bg": bg, "bu": bu, "bd": bd,
            "cosk": cosk, "sink": sink, "cosq": cosq, "sinq": sinq,
            "mask4": mask4, "onesb": onesb_np, "epsv": epsv,
        })
    return in_maps


def kernel(**inputs):
    global LAST_RESULT
    if "nc" not in _CACHE:
        _CACHE["nc"] = _build_program()
    nc = _CACHE["nc"]
    in_maps = _prepare_inputs(inputs)
    trace = bool(int(os.environ.get("BASS_TRACE", "0")))
    res = run_bass_kernel_spmd(nc, in_maps, core_ids=list(range(NC)),
                               trace=trace)
    LAST_RESULT = res
    # assemble: core c owns query chunks {p+4i} of batch c//4 (p = c%4)
    full = np.empty((B * S, D), np.float32)
    for c in range(NC):
        b, p = c // 4, c % 4
        oc = res.results[c]["out"]                      # [D, TOK]
        for i in range(4):
            qc = p + 4 * i
            full[b * S + qc * P:b * S + (qc + 1) * P, :] = \
                oc[:, i * P:(i + 1) * P].T
    return full.reshape(B, S, D)


if __name__ == "__main__":
    print("import as module; use kernel(**inputs)")


# revision 17
# speedup vs baseline: 1.1733x; 1.1733x over previous
"""Trainium2 Bass kernel for a Llama block (B=2, S=2048, D=2048, H=16, FF=8192).

Sharding (8 cores, fully static SPMD — one program, per-core differences are
input data only):
  - core c (batch b = c//4, p = c%4) owns query chunks {p, p+4, p+8, p+12}
    (each 128 tokens) of its batch: Q projection, attention queries, WO,
    norm2 and the FFN for those 512 tokens.
  - core c computes K/V projections for own heads [4p, 4p+4) of its batch;
    four group-of-4 AllGathers (one per own head j, K+V together) pipeline
    against Q projection and the attention rounds: after AG_j the core holds
    heads {j, 4+j, 8+j, 12+j}, processed as attention round j.
  - attention is causal with uniform suffix widths: at key block kb only
    query slots i >= kb//4 are live; the first live slot gets an additive
    mask selected by kb%4 from a per-core [128, 4, 128] table (zeros /
    triangular / -inf), so the instruction stream is core-invariant.
  - normalized activations nx = rmsnorm(x) of the whole batch are computed
    once into SBUF (bf16, 64 KB/partition) and reused by K and V.
  - FFN runs in fp8 (e4m3, weights host-scaled x16) with DoubleRow matmuls.
  - activations live in [feature, token] layout throughout; the host passes x
    pre-transposed and reassembles the transposed per-core outputs.
  - RMSNorm weights are folded into the following projection weights on the
    host (exact here since they are 1.0, and algebraically valid in general).
"""

import contextlib
import math
import os
import sys
import types

sys.path.insert(0, "/opt/trn_rl_repo")


def _install_ntff_hook_shim():
    """The image's antenv package lacks axon_hooks; provide it so
    run_bass_kernel_spmd(trace=True) can fetch the NTFF profile hook.
    Degrades silently if the boot .so lacks the profiling symbols."""
    if "antenv.axon_hooks" in sys.modules:
        return
    mod = types.ModuleType("antenv.axon_hooks")
    mod._HOOK = None
    mod.set_axon_ntff_profile_hook = lambda h: setattr(mod, "_HOOK", h)
    mod.get_axon_ntff_profile_hook = lambda: mod._HOOK
    sys.modules["antenv.axon_hooks"] = mod
    try:
        if "/root/.axon_site" not in sys.path:
            sys.path.append("/root/.axon_site")
        from trn_agent_boot.trn_boot import _ntff_profile_via_ctypes

        mod._HOOK = _ntff_profile_via_ctypes("/opt/axon/libaxon_pjrt.so")
    except Exception:
        pass


_install_ntff_hook_shim()

import ml_dtypes
import numpy as np

import concourse.bass as bass
import concourse.mybir as mybir
import concourse.tile as tile
from concourse import bacc
from concourse.bass_utils import run_bass_kernel_spmd

F32 = mybir.dt.float32
F32R = mybir.dt.float32r
BF16 = mybir.dt.bfloat16
F8 = mybir.dt.float8e4
AFT = mybir.ActivationFunctionType
ALU = mybir.AluOpType
DR = mybir.MatmulPerfMode.DoubleRow
W8SCALE = 16.0          # fp8 FFN weights are host-scaled by this

B, S, D, H = 2, 2048, 2048, 16
HD = D // H            # 128
FF = 4 * D             # 8192
NC = 8
TOK = 512              # own tokens per core
EPS = 1e-6
BASE = 10000.0
NEG = -1e30
P = 128
DCH = D // P           # 16 d-chunks
FCH = FF // P          # 64 ff subchunks
SCALE = 1.0 / math.sqrt(HD)
KB = S // P            # 16 key blocks
HDS = HD * S           # one head's K (or V) element count

_CACHE = {}
LAST_RESULT = None


def _rope_tables(positions):
    """[128, n] cos/sin tables with the 64-row table duplicated in both
    partition halves (for lane-aligned rope on-device)."""
    t = BASE ** (-2.0 * (np.arange(HD // 2, dtype=np.float64) - 1.0) / HD)
    ang = positions[:, None].astype(np.float64) * t[None, :]       # [n, 64]
    c = np.cos(ang).T.astype(np.float32)
    sn = np.sin(ang).T.astype(np.float32)
    return (np.concatenate([c, c], axis=0), np.concatenate([sn, sn], axis=0))


def _build_program():
    nc = bacc.Bacc("TRN2", target_bir_lowering=False, debug=False,
                   num_devices=NC)

    def inp(name, shape, dtype=F32R):
        return nc.dram_tensor(name, shape, dtype, kind="ExternalInput").ap()

    xT_b = inp("xT_b", [D, S])             # own batch, transposed
    xT_own = inp("xT_own", [D, TOK])       # own query chunks, transposed
    wq = inp("wq", [H, P, DCH, HD], BF16)      # pre-tiled [h, p, o, f]
    wk = inp("wk", [D, 4 * HD], BF16)
    wv = inp("wv", [D, 4 * HD], BF16)
    wo = inp("wo", [DCH, P, DCH, P], BF16)     # pre-tiled [o, p, a, f]
    wg = inp("wg", [FCH, P, DCH, P], F8)       # pre-tiled [fb, p, o, f], x16
    wu = inp("wu", [FCH, P, DCH, P], F8)       # pre-tiled [fb, p, o, f], x16
    wd = inp("wd", [4, DCH, P, DCH, P], F8)    # pre-tiled [sc, o, p, fs, f], x16
    bq = inp("bq", [P, H], F32)
    bk = inp("bk", [P, 4], F32)
    bvb = inp("bvb", [P, 4 * HD], F32)
    bo = inp("bo", [P, DCH], F32)
    bg = inp("bg", [P, FCH], F32)
    bu = inp("bu", [P, FCH], F32)
    bd = inp("bd", [P, DCH], F32)
    cosk = inp("cosk", [P, S], F32)
    sink = inp("sink", [P, S], F32)
    cosq = inp("cosq", [P, TOK], F32)
    sinq = inp("sinq", [P, TOK], F32)
    mask4 = inp("mask4", [P, 4, P], F32)   # per-core additive causal masks
    onesb = inp("onesb", [P, 1], BF16)
    epsv = inp("epsv", [P, 1], F32)
    out_t = nc.dram_tensor("out", [D, TOK], F32, kind="ExternalOutput").ap()

    xT_b3 = xT_b.rearrange("(o p) t -> p o t", p=P)
    xT_own3 = xT_own.rearrange("(o p) t -> p o t", p=P)

    with tile.TileContext(nc) as tc:
        with tc.tile_pool(name="consts", bufs=1) as consts, \
             tc.tile_pool(name="big", bufs=1) as big, \
             tc.tile_pool(name="dram", bufs=1, space="DRAM") as dram:
            onesb_s = consts.tile([P, 1], BF16)
            nc.sync.dma_start(onesb_s[:], onesb[:])
            eps_s = consts.tile([P, 1], F32)
            nc.sync.dma_start(eps_s[:], epsv[:])
            bq_s = consts.tile([P, H], F32)
            nc.sync.dma_start(bq_s[:], bq[:])
            bk_s = consts.tile([P, 4], F32)
            nc.sync.dma_start(bk_s[:], bk[:])
            bvb_s = consts.tile([P, 4 * HD], F32)
            nc.sync.dma_start(bvb_s[:], bvb[:])
            bo_s = consts.tile([P, DCH], F32)
            nc.sync.dma_start(bo_s[:], bo[:])
            bg_s = consts.tile([P, FCH], F32)
            nc.sync.dma_start(bg_s[:], bg[:])
            bu_s = consts.tile([P, FCH], F32)
            nc.sync.dma_start(bu_s[:], bu[:])
            bd_s = consts.tile([P, DCH], F32)
            nc.sync.dma_start(bd_s[:], bd[:])
            mask_s = consts.tile([P, 4, P], F32)
            nc.sync.dma_start(mask_s[:], mask4[:])
            cosq_s = consts.tile([P, TOK], F32)
            nc.sync.dma_start(cosq_s[:], cosq[:])
            sinq_s = consts.tile([P, TOK], F32)
            nc.sync.dma_start(sinq_s[:], sinq[:])

            # per-own-head K+V bounce (1 MB) and gather (4 MB) buffers
            kv_bounce = [dram.tile([2 * HDS], BF16, name=f"kvb{j}")
                         for j in range(4)]
            kv_gath = [dram.tile([4 * 2 * HDS], BF16, name=f"kvg{j}")
                       for j in range(4)]
            groups = [[0, 1, 2, 3], [4, 5, 6, 7]]

            # big SBUF slots (aliased across phases by tag)
            #   A (64K): nx_all (P1-P2) -> attention round K/V double-buffer
            #   B (32K): kv_ws (P2) -> acc (P5-P7)
            #   C (16K): cskt rope tables (P2) -> att_all (P4-P5) -> act2 (P6)
            #   D (16K): qt_all (P3-P4) -> nx2 (P6)
            nx_all = big.tile([P, DCH, S], BF16, tag="bigA", name="nx_all")
            kv_ws = big.tile([P, 2, DCH, 4 * HD], BF16, tag="bigB",
                             name="kv_ws")
            cskt = big.tile([P, 2, S], F32, tag="bigC", name="cskt")
            nc.sync.dma_start(cskt[:, 0, :], cosk[:])
            nc.sync.dma_start(cskt[:, 1, :], sink[:])
            nc.sync.dma_start(kv_ws[:, 0],
                              wk.rearrange("(o p) f -> p o f", p=P))
            nc.sync.dma_start(kv_ws[:, 1],
                              wv.rearrange("(o p) f -> p o f", p=P))

            def rope(pool, src, cos_t, sin_t, dst, tname):
                """src [128, n] f32 pre-rope -> dst [128, n] roped.

                cos_t/sin_t are [128, n] with the 64-row table duplicated in
                both partition halves. A half-swapped copy of src keeps every
                elementwise op lane-aligned:
                  ma = src*cos  -> [f*cos ; s*cos]
                  mb = swap(src)*sin -> [s*sin ; f*sin]
                  dst = [ma_top + mb_top ; mb_bot - ma_bot]
                """
                n = src.shape[-1]
                hh = HD // 2
                swp = pool.tile([P, n], F32, tag="rpsw", name=f"{tname}sw")
                nc.sync.dma_start(swp[0:hh, :], src[hh:P, :])
                nc.sync.dma_start(swp[hh:P, :], src[0:hh, :])
                ma = pool.tile([P, n], F32, tag="rp1", name=f"{tname}ma")
                mb = pool.tile([P, n], F32, tag="rp2", name=f"{tname}mb")
                nc.vector.tensor_mul(out=ma[:], in0=src[:], in1=cos_t)
                nc.vector.tensor_mul(out=mb[:], in0=swp[:], in1=sin_t)
                nc.vector.tensor_add(out=dst[0:hh], in0=ma[0:hh],
                                     in1=mb[0:hh])
                nc.vector.tensor_sub(out=dst[hh:P], in0=mb[hh:P],
                                     in1=ma[hh:P])

            # ---- P1: load x once, rmsnorm into SBUF (nx_all, bf16) ----
            with tc.tile_pool(name="p1", bufs=3) as pool, \
                 tc.tile_pool(name="p1ps", bufs=2, space="PSUM") as psum:
                for tb in range(S // TOK):
                    cols = bass.ts(tb, TOK)
                    sumsq = psum.tile([1, TOK], F32, tag="n1ss",
                                      name=f"n1ss{tb}")
                    for o in range(DCH):
                        xc = pool.tile([P, TOK], F32R, tag="n1x",
                                       name=f"n1x{tb}_{o}")
                        nc.sync.dma_start(xc[:], xT_b3[:, o, cols])
                        sq = pool.tile([P, TOK], BF16, tag="n1sq",
                                       name=f"n1sq{tb}_{o}")
                        nc.scalar.activation(sq[:], xc[:], AFT.Square)
                        nc.tensor.matmul(sumsq[:], lhsT=onesb_s[:], rhs=sq[:],
                                         start=(o == 0), stop=(o == DCH - 1))
                    rms = pool.tile([1, TOK], F32, tag="n1rms",
                                    name=f"n1rms{tb}")
                    nc.scalar.activation(rms[:], sumsq[:], AFT.Sqrt,
                                         scale=1.0 / D, bias=eps_s[:1])
                    rec = pool.tile([1, TOK], F32, tag="n1rec",
                                    name=f"n1rec{tb}")
                    nc.vector.reciprocal(rec[:], rms[:])
                    rbc = pool.tile([P, TOK], F32, tag="n1rbc",
                                    name=f"n1rbc{tb}")
                    nc.gpsimd.partition_broadcast(rbc[:], rec[:])
                    for o in range(DCH):
                        xc2 = pool.tile([P, TOK], F32R, tag="n1x2",
                                        name=f"n1x2{tb}_{o}")
                        nc.sync.dma_start(xc2[:], xT_b3[:, o, cols])
                        nc.vector.tensor_mul(out=nx_all[:, o, cols],
                                             in0=xc2[:].bitcast(F32),
                                             in1=rbc[:])

            # ---- P2a: V projections for own 4 heads (wide rhs) ----
            with tc.tile_pool(name="p2a", bufs=2) as pool, \
                 tc.tile_pool(name="p2aps", bufs=2, space="PSUM") as psum:
                for tb in range(S // TOK):
                    vps = [psum.tile([P, 4 * HD], F32, tag=f"vps{i}",
                                     name=f"vps{i}_{tb}") for i in range(4)]
                    for o in range(DCH):
                        st, sp = (o == 0), (o == DCH - 1)
                        for ts_ in range(4):
                            nc.tensor.matmul(
                                vps[ts_][:],
                                lhsT=nx_all[:, o,
                                            bass.ds(tb * TOK + ts_ * P, P)],
                                rhs=kv_ws[:, 1, o, :], start=st, stop=sp)
                    for ts_ in range(4):
                        vsb = pool.tile([P, 4 * HD], BF16, tag="vsb",
                                        name=f"vsb{tb}_{ts_}")
                        nc.vector.tensor_add(out=vsb[:], in0=vps[ts_][:],
                                             in1=bvb_s[:])
                        row0 = tb * TOK + ts_ * P
                        for j in range(4):
                            vdst = kv_bounce[j][HDS:2 * HDS].rearrange(
                                "(s hd) -> s hd", hd=HD)
                            nc.sync.dma_start(
                                vdst[bass.ds(row0, P), :],
                                vsb[:, bass.ts(j, HD)])

            # ---- P2b: K proj per own head j + rope -> AG_j (K+V) ----
            with tc.tile_pool(name="p2b", bufs=2) as pool, \
                 tc.tile_pool(name="p2bps", bufs=2, space="PSUM") as psum:
                for j in range(4):
                    kdst = kv_bounce[j][0:HDS].rearrange("(hd s) -> hd s", s=S)
                    for tb in range(S // TOK):
                        cols = bass.ts(tb, TOK)
                        kps = psum.tile([P, TOK], F32, tag="kps",
                                        name=f"kps{j}_{tb}")
                        for o in range(DCH):
                            nc.tensor.matmul(
                                kps[:], lhsT=kv_ws[:, 0, o, bass.ts(j, HD)],
                                rhs=nx_all[:, o, cols],
                                start=(o == 0), stop=(o == DCH - 1))
                        kb_t = pool.tile([P, TOK], F32, tag="kbias",
                                         name=f"kbias{j}_{tb}")
                        nc.scalar.activation(kb_t[:], kps[:], AFT.Identity,
                                             bias=bk_s[:, j:j + 1])
                        krt = pool.tile([P, TOK], BF16, tag="kroped",
                                        name=f"kroped{j}_{tb}")
                        rope(pool, kb_t[:], cskt[:, 0, cols], cskt[:, 1, cols],
                             krt[:], f"kr{j}_{tb}")
                        nc.sync.dma_start(kdst[:, cols], krt[:])
                    nc.gpsimd.collective_compute(
                        "AllGather", mybir.AluOpType.bypass,
                        ins=[kv_bounce[j][:].opt()],
                        outs=[kv_gath[j][:].opt()],
                        replica_groups=groups)

            # ---- P3: Q projections for own tokens, all heads + rope ----
            qt_all = big.tile([P, H, TOK], BF16, tag="bigD", name="qt_all")
            with tc.tile_pool(name="p3", bufs=2) as pool, \
                 tc.tile_pool(name="p3q", bufs=1) as poolq, \
                 tc.tile_pool(name="p3ps", bufs=2, space="PSUM") as psum:
                nxq = poolq.tile([P, DCH, TOK], BF16, tag="nxq")
                rbcq = poolq.tile([P, TOK], F32, tag="rbcq")
                sumsq = psum.tile([1, TOK], F32, tag="qss")
                for o in range(DCH):
                    xc = pool.tile([P, TOK], F32R, tag="qx", name=f"qx{o}")
                    nc.sync.dma_start(xc[:], xT_own3[:, o, :])
                    sq = pool.tile([P, TOK], BF16, tag="qsq", name=f"qsq{o}")
                    nc.scalar.activation(sq[:], xc[:], AFT.Square)
                    nc.tensor.matmul(sumsq[:], lhsT=onesb_s[:], rhs=sq[:],
                                     start=(o == 0), stop=(o == DCH - 1))
                rms = pool.tile([1, TOK], F32, tag="qrms")
                nc.scalar.activation(rms[:], sumsq[:], AFT.Sqrt,
                                     scale=1.0 / D, bias=eps_s[:1])
                rec = pool.tile([1, TOK], F32, tag="qrec")
                nc.vector.reciprocal(rec[:], rms[:])
                nc.gpsimd.partition_broadcast(rbcq[:], rec[:])
                for o in range(DCH):
                    xc2 = pool.tile([P, TOK], F32R, tag="qx2", name=f"qx2{o}")
                    nc.sync.dma_start(xc2[:], xT_own3[:, o, :])
                    nc.vector.tensor_mul(out=nxq[:, o, :],
                                         in0=xc2[:].bitcast(F32),
                                         in1=rbcq[:])
                for h in range(H):
                    wq_s = pool.tile([P, DCH, HD], BF16, tag="wqs",
                                     name=f"wqs{h}")
                    nc.sync.dma_start(wq_s[:], wq[h])
                    qp = psum.tile([P, TOK], F32, tag="qps", name=f"qps{h}")
                    for o in range(DCH):
                        nc.tensor.matmul(qp[:], lhsT=wq_s[:, o, :],
                                         rhs=nxq[:, o, :],
                                         start=(o == 0),
                                         stop=(o == DCH - 1))
                    qb_t = pool.tile([P, TOK], F32, tag="qbias",
                                     name=f"qbias{h}")
                    nc.scalar.activation(qb_t[:], qp[:], AFT.Identity,
                                         bias=bq_s[:, h:h + 1])
                    rope(pool, qb_t[:], cosq_s[:], sinq_s[:],
                         qt_all[:, h, :], f"qr{h}")

            # ---- P4: attention rounds (causal, stride-4 query chunks) ----
            # round j processes heads {4m+j}; K/V from AG_j.  At key block kb
            # only query slots i >= kb//4 are live (suffix); the first live
            # slot gets the additive mask mask_s[:, kb%4, :].
            att_all = big.tile([P, H, TOK], BF16, tag="bigC", name="att_all")
            kvr = big.tile([P, 2, 4, 2, S], BF16, tag="bigA", name="kvr")
            with tc.tile_pool(name="p4", bufs=3) as pool, \
                 tc.tile_pool(name="p4ps", bufs=2, space="PSUM") as psum:
                for j in range(4):
                    r2 = j % 2
                    for m in range(4):
                        h = 4 * m + j
                        ktg = kvr[:, r2, m, 0, :]
                        nc.sync.dma_start(
                            ktg,
                            kv_gath[j][bass.ds(m * 2 * HDS, HDS)].rearrange(
                                "(hd s) -> hd s", s=S))
                        vg = kvr[:, r2, m, 1, :].rearrange(
                            "p (kb hd) -> p kb hd", hd=HD)
                        nc.sync.dma_start(
                            vg,
                            kv_gath[j][bass.ds(m * 2 * HDS + HDS,
                                               HDS)].rearrange(
                                "(kb p hd) -> p kb hd", p=P, hd=HD))
                        den = psum.tile([1, TOK], F32, tag="denps",
                                        name=f"den{h}")
                        op = psum.tile([P, TOK], F32, tag="outps",
                                       name=f"ops{h}")
                        for kb in range(KB):
                            off = (kb // 4) * P
                            stp = psum.tile([P, TOK], F32, tag="stps",
                                            name=f"st{h}_{kb}")
                            nc.tensor.matmul(stp[:, off:TOK],
                                             lhsT=ktg[:, bass.ts(kb, P)],
                                             rhs=qt_all[:, h, off:TOK],
                                             start=True, stop=True)
                            nc.vector.tensor_add(out=stp[:, off:off + P],
                                                 in0=stp[:, off:off + P],
                                                 in1=mask_s[:, kb % 4, :])
                            est = pool.tile([P, TOK], BF16, tag="est",
                                            name=f"est{h}_{kb}")
                            nc.scalar.activation(est[:, off:TOK],
                                                 stp[:, off:TOK],
                                                 AFT.Exp, scale=SCALE)
                            st, sp = (kb == 0), (kb == KB - 1)
                            nc.tensor.matmul(den[:, off:TOK], lhsT=onesb_s[:],
                                             rhs=est[:, off:TOK],
                                             start=st, stop=sp,
                                             skip_group_check=True)
                            nc.tensor.matmul(op[:, off:TOK],
                                             lhsT=vg[:, kb, :],
                                             rhs=est[:, off:TOK],
                                             start=st, stop=sp,
                                             skip_group_check=True)
                        recd = pool.tile([1, TOK], F32, tag="recd",
                                         name=f"recd{h}")
                        nc.vector.reciprocal(recd[:], den[:])
                        rdb = pool.tile([P, TOK], F32, tag="rdb",
                                        name=f"rdb{h}")
                        nc.gpsimd.partition_broadcast(rdb[:], recd[:])
                        nc.vector.tensor_mul(out=att_all[:, h, :], in0=op[:],
                                             in1=rdb[:])

            # ---- P5: WO for own tokens + residual -> acc (= x2T) ----
            acc = big.tile([P, DCH, TOK], F32, tag="bigB", name="acc")
            with tc.tile_pool(name="p5", bufs=2) as pool, \
                 tc.tile_pool(name="p5ps", bufs=2, space="PSUM") as psum:
                for o in range(DCH):
                    wo_s = pool.tile([P, DCH, P], BF16, tag="wos",
                                     name=f"wos{o}")
                    nc.sync.dma_start(wo_s[:], wo[o])
                    x2p = psum.tile([P, TOK], F32, tag="x2ps", name=f"x2ps{o}")
                    for h in range(H):
                        nc.tensor.matmul(x2p[:], lhsT=wo_s[:, h, :],
                                         rhs=att_all[:, h, :],
                                         start=(h == 0), stop=(h == H - 1))
                    x2pre = pool.tile([P, TOK], F32, tag="x2pre",
                                      name=f"x2pre{o}")
                    nc.scalar.activation(x2pre[:], x2p[:], AFT.Identity,
                                         bias=bo_s[:, o:o + 1])
                    xres = pool.tile([P, TOK], F32R, tag="xres",
                                     name=f"xres{o}")
                    nc.sync.dma_start(xres[:], xT_own3[:, o, :])
                    nc.vector.tensor_add(out=acc[:, o, :], in0=x2pre[:],
                                         in1=xres[:].bitcast(F32))

            # ---- P6: norm2 + FFN (fp8 weights/activations, DoubleRow) ----
            nx2 = big.tile([P, DCH, TOK], F8, tag="bigD", name="nx2")
            act2 = big.tile([P, 2, DCH, TOK], F8, tag="bigC", name="act2")
            with tc.tile_pool(name="p6w", bufs=3) as wpool6, \
                 tc.tile_pool(name="p6", bufs=2) as pool, \
                 tc.tile_pool(name="p6ps", bufs=2, space="PSUM") as psum:
                rbc2 = pool.tile([P, TOK], F32, tag="rbc2")
                sumsq = psum.tile([1, TOK], F32, tag="n2ss")
                for o in range(DCH):
                    sq = pool.tile([P, TOK], BF16, tag="n2sq", name=f"n2sq{o}")
                    nc.scalar.activation(sq[:], acc[:, o, :], AFT.Square)
                    nc.tensor.matmul(sumsq[:], lhsT=onesb_s[:], rhs=sq[:],
                                     start=(o == 0), stop=(o == DCH - 1))
                rms = pool.tile([1, TOK], F32, tag="n2rms")
                nc.scalar.activation(rms[:], sumsq[:], AFT.Sqrt,
                                     scale=1.0 / D, bias=eps_s[:1])
                rec = pool.tile([1, TOK], F32, tag="n2rec")
                nc.vector.reciprocal(rec[:], rms[:])
                nc.gpsimd.partition_broadcast(rbc2[:], rec[:])
                for o in range(DCH):
                    nc.vector.tensor_mul(out=nx2[:, o, :], in0=acc[:, o, :],
                                         in1=rbc2[:])
                # fold b_down into acc now (added once)
                for o in range(DCH):
                    nc.vector.tensor_scalar_add(acc[:, o, :], acc[:, o, :],
                                                bd_s[:, o:o + 1])
                for sc in range(4):
                    for fs in range(DCH):
                        f = sc * DCH + fs
                        wg_s = wpool6.tile([P, DCH, P], F8, tag="wgs",
                                           name=f"wgs{f}")
                        nc.sync.dma_start(wg_s[:], wg[f])
                        wu_s = wpool6.tile([P, DCH, P], F8, tag="wus",
                                           name=f"wus{f}")
                        nc.sync.dma_start(wu_s[:], wu[f])
                        gp = psum.tile([P, TOK], F32, tag="gps", name=f"gps{f}")
                        up = psum.tile([P, TOK], F32, tag="ups", name=f"ups{f}")
                        for oj in range(DCH // 2):
                            st, sp = (oj == 0), (oj == DCH // 2 - 1)
                            o2 = bass.ds(2 * oj, 2)
                            nc.tensor.matmul(gp[:], lhsT=wg_s[:, o2, :],
                                             rhs=nx2[:, o2, :], start=st,
                                             stop=sp, perf_mode=DR)
                            nc.tensor.matmul(up[:], lhsT=wu_s[:, o2, :],
                                             rhs=nx2[:, o2, :], start=st,
                                             stop=sp, perf_mode=DR)
                        gs = pool.tile([P, TOK], F32, tag="gsig", name=f"gs{f}")
                        nc.scalar.activation(gs[:], gp[:], AFT.Silu,
                                             scale=1.0 / W8SCALE,
                                             bias=bg_s[:, f:f + 1])
                        us = pool.tile([P, TOK], F32, tag="usig", name=f"us{f}")
                        nc.scalar.activation(us[:], up[:], AFT.Identity,
                                             scale=1.0 / W8SCALE,
                                             bias=bu_s[:, f:f + 1])
                        nc.vector.tensor_mul(out=act2[:, sc % 2, fs, :],
                                             in0=gs[:], in1=us[:])
                    for o in range(DCH):
                        wd_s = wpool6.tile([P, DCH, P], F8, tag="wds",
                                           name=f"wds{sc}_{o}")
                        nc.sync.dma_start(wd_s[:], wd[sc, o])
                        dp = psum.tile([P, TOK], F32, tag="dps",
                                       name=f"dps{sc}_{o}")
                        for fj in range(DCH // 2):
                            f2 = bass.ds(2 * fj, 2)
                            nc.tensor.matmul(dp[:], lhsT=wd_s[:, f2, :],
                                             rhs=act2[:, sc % 2, f2, :],
                                             start=(fj == 0),
                                             stop=(fj == DCH // 2 - 1),
                                             perf_mode=DR)
                        nc.vector.scalar_tensor_tensor(
                            out=acc[:, o, :], in0=dp[:],
                            scalar=1.0 / W8SCALE, in1=acc[:, o, :],
                            op0=ALU.mult, op1=ALU.add)

            # ---- P7: write transposed output ----
            nc.sync.dma_start(
                out_t.rearrange("(o p) t -> p o t", p=P), acc[:])

    nc.compile()
    return nc


def _prepare_inputs(inputs):
    """Build the 8 per-core in_maps from the full problem inputs."""
    x = np.ascontiguousarray(inputs["x"], dtype=np.float32)   # [B, S, D]
    n1 = np.asarray(inputs["norm1_w"], dtype=np.float32)
    n2 = np.asarray(inputs["norm2_w"], dtype=np.float32)
    wq_f = np.ascontiguousarray(n1[:, None] * np.asarray(inputs["wq"], np.float32))
    wk_f = n1[:, None] * np.asarray(inputs["wk"], np.float32)
    wv_f = n1[:, None] * np.asarray(inputs["wv"], np.float32)
    wo_f = np.ascontiguousarray(np.asarray(inputs["wo"], np.float32))
    wg_f = np.ascontiguousarray(n2[:, None] * np.asarray(inputs["w_gate"], np.float32))
    wu_f = np.ascontiguousarray(n2[:, None] * np.asarray(inputs["w_up"], np.float32))
    wd_f = np.ascontiguousarray(np.asarray(inputs["w_down"], np.float32))
    bq = np.asarray(inputs["bq"], np.float32).reshape(H, P).T.copy()
    bo = np.asarray(inputs["bo"], np.float32).reshape(DCH, P).T.copy()
    bg = np.asarray(inputs["b_gate"], np.float32).reshape(FCH, P).T.copy()
    bu = np.asarray(inputs["b_up"], np.float32).reshape(FCH, P).T.copy()
    bd = np.asarray(inputs["b_down"], np.float32).reshape(DCH, P).T.copy()
    bk_full = np.asarray(inputs["bk"], np.float32)
    bv_full = np.asarray(inputs["bv"], np.float32)

    cosk, sink = _rope_tables(np.arange(S))
    onesb_np = np.ones((P, 1), ml_dtypes.bfloat16)
    epsv = np.full((P, 1), EPS, np.float32)

    xT = [np.ascontiguousarray(x[b].T) for b in range(B)]      # [D, S]
    bf = ml_dtypes.bfloat16
    # pre-tiled layouts so every weight-tile DMA is one contiguous block
    wq_b = np.ascontiguousarray(
        wq_f.astype(bf).reshape(DCH, P, H, HD).transpose(2, 1, 0, 3))
    wk_b = wk_f.astype(bf)
    wv_b = wv_f.astype(bf)
    wo_b = np.ascontiguousarray(
        wo_f.astype(bf).reshape(DCH, P, DCH, P).transpose(2, 1, 0, 3))
    f8 = ml_dtypes.float8_e4m3
    wg_b = np.ascontiguousarray(
        (W8SCALE * wg_f).astype(f8).reshape(DCH, P, FCH, P).transpose(2, 1, 0, 3))
    wu_b = np.ascontiguousarray(
        (W8SCALE * wu_f).astype(f8).reshape(DCH, P, FCH, P).transpose(2, 1, 0, 3))
    wd_b = np.ascontiguousarray(
        (W8SCALE * wd_f).astype(f8).reshape(4, DCH, P, DCH, P).transpose(0, 3, 2, 1, 4))

    tri = np.where(np.arange(P)[:, None] > np.arange(P)[None, :],
                   NEG, 0.0).astype(np.float32)
    in_maps = []
    for c in range(NC):
        b, p = c // 4, c % 4
        kv0 = 4 * p * HD                                        # head-col base
        # owned query chunks: slots i = 0..3 hold chunk p + 4i (128 tokens)
        qpos = np.concatenate([np.arange(P) + (p + 4 * i) * P
                               for i in range(4)])
        cosq, sinq = _rope_tables(qpos)
        mask4 = np.empty((P, 4, P), np.float32)
        for j in range(4):
            if j < p:
                mask4[:, j, :] = 0.0
            elif j == p:
                mask4[:, j, :] = tri
            else:
                mask4[:, j, :] = NEG
        in_maps.append({
            "xT_b": xT[b],
            "xT_own": np.ascontiguousarray(xT[b][:, qpos]),
            "wq": wq_b,
            "wk": np.ascontiguousarray(wk_b[:, kv0:kv0 + 4 * HD]),
            "wv": np.ascontiguousarray(wv_b[:, kv0:kv0 + 4 * HD]),
            "wo": wo_b,
            "wg": wg_b, "wu": wu_b, "wd": wd_b,
            "bq": bq,
            "bk": bk_full[kv0:kv0 + 4 * HD].reshape(4, P).T.copy(),
            "bvb": np.tile(bv_full[kv0:kv0 + 4 * HD][None, :], (P, 1)).copy(),
            "bo": bo, "bg": bg, "bu": bu, "bd": bd,
            "cosk": cosk, "sink": sink, "cosq": cosq, "sinq": sinq,
            "mask4": mask4, "onesb": onesb_np, "epsv": epsv,
        })
    return in_maps


def kernel(**inputs):
    global LAST_RESULT
    if "nc" not in _CACHE:
        _CACHE["nc"] = _build_program()
    nc = _CACHE["nc"]
    in_maps = _prepare_inputs(inputs)
    trace = bool(int(os.environ.get("BASS_TRACE", "0")))
    res = run_bass_kernel_spmd(nc, in_maps, core_ids=list(range(NC)),
                               trace=trace)
    LAST_RESULT = res
    # assemble: core c owns query chunks {p+4i} of batch c//4 (p = c%4)
    full = np.empty((B * S, D), np.float32)
    for c in range(NC):
        b, p = c // 4, c % 4
        oc = res.results[c]["out"]                      # [D, TOK]
        for i in range(4):
            qc = p + 4 * i
            full[b * S + qc * P:b * S + (qc + 1) * P, :] = \
                oc[:, i * P:(i + 1) * P].T
    return full.reshape(B, S, D)


if __name__ == "__main__":
    print("import as module; use kernel(**inputs)")


# revision 18
# speedup vs baseline: 1.2024x; 1.0248x over previous
"""Trainium2 Bass kernel for a Llama block (B=2, S=2048, D=2048, H=16, FF=8192).

Sharding (8 cores, fully static SPMD — one program, per-core differences are
input data only):
  - core c (batch b = c//4, p = c%4) owns query chunks {p, p+4, p+8, p+12}
    (each 128 tokens) of its batch: Q projection, attention queries, WO,
    norm2 and the FFN for those 512 tokens.
  - core c computes K/V projections for own heads [4p, 4p+4) of its batch;
    four group-of-4 AllGathers (one per own head j, K+V together) pipeline
    against Q projection and the attention rounds: after AG_j the core holds
    heads {j, 4+j, 8+j, 12+j}, processed as attention round j.
  - attention is causal with uniform suffix widths: at key block kb only
    query slots i >= kb//4 are live; the first live slot gets an additive
    mask selected by kb%4 from a per-core [128, 4, 128] table (zeros /
    triangular / -inf), so the instruction stream is core-invariant.
  - normalized activations nx = rmsnorm(x) of the whole batch are computed
    once into SBUF (bf16, 64 KB/partition) and reused by K and V.
  - FFN runs in fp8 (e4m3, weights host-scaled x16) with DoubleRow matmuls.
  - activations live in [feature, token] layout throughout; the host passes x
    pre-transposed and reassembles the transposed per-core outputs.
  - RMSNorm weights are folded into the following projection weights on the
    host (exact here since they are 1.0, and algebraically valid in general).
"""

import contextlib
import math
import os
import sys
import types

sys.path.insert(0, "/opt/trn_rl_repo")


def _install_ntff_hook_shim():
    """The image's antenv package lacks axon_hooks; provide it so
    run_bass_kernel_spmd(trace=True) can fetch the NTFF profile hook.
    Degrades silently if the boot .so lacks the profiling symbols."""
    if "antenv.axon_hooks" in sys.modules:
        return
    mod = types.ModuleType("antenv.axon_hooks")
    mod._HOOK = None
    mod.set_axon_ntff_profile_hook = lambda h: setattr(mod, "_HOOK", h)
    mod.get_axon_ntff_profile_hook = lambda: mod._HOOK
    sys.modules["antenv.axon_hooks"] = mod
    try:
        if "/root/.axon_site" not in sys.path:
            sys.path.append("/root/.axon_site")
        from trn_agent_boot.trn_boot import _ntff_profile_via_ctypes

        mod._HOOK = _ntff_profile_via_ctypes("/opt/axon/libaxon_pjrt.so")
    except Exception:
        pass


_install_ntff_hook_shim()

import ml_dtypes
import numpy as np

import concourse.bass as bass
import concourse.mybir as mybir
import concourse.tile as tile
from concourse import bacc
from concourse.bass_utils import run_bass_kernel_spmd

F32 = mybir.dt.float32
F32R = mybir.dt.float32r
BF16 = mybir.dt.bfloat16
F8 = mybir.dt.float8e4
AFT = mybir.ActivationFunctionType
ALU = mybir.AluOpType
DR = mybir.MatmulPerfMode.DoubleRow
W8SCALE = 16.0          # fp8 FFN weights are host-scaled by this

B, S, D, H = 2, 2048, 2048, 16
HD = D // H            # 128
FF = 4 * D             # 8192
NC = 8
TOK = 512              # own tokens per core
EPS = 1e-6
BASE = 10000.0
NEG = -1e30
P = 128
DCH = D // P           # 16 d-chunks
FCH = FF // P          # 64 ff subchunks
SCALE = 1.0 / math.sqrt(HD)
KB = S // P            # 16 key blocks
HDS = HD * S           # one head's K (or V) element count

_CACHE = {}
LAST_RESULT = None


def _rope_tables(positions):
    """[128, n] cos/sin tables with the 64-row table duplicated in both
    partition halves (for lane-aligned rope on-device)."""
    t = BASE ** (-2.0 * (np.arange(HD // 2, dtype=np.float64) - 1.0) / HD)
    ang = positions[:, None].astype(np.float64) * t[None, :]       # [n, 64]
    c = np.cos(ang).T.astype(np.float32)
    sn = np.sin(ang).T.astype(np.float32)
    return (np.concatenate([c, c], axis=0), np.concatenate([sn, sn], axis=0))


def _build_program():
    nc = bacc.Bacc("TRN2", target_bir_lowering=False, debug=False,
                   num_devices=NC)

    def inp(name, shape, dtype=F32R):
        return nc.dram_tensor(name, shape, dtype, kind="ExternalInput").ap()

    xT_b = inp("xT_b", [D, S])             # own batch, transposed
    xT_own = inp("xT_own", [D, TOK])       # own query chunks, transposed
    wq = inp("wq", [H, P, DCH, HD], BF16)      # pre-tiled [h, p, o, f]
    wk = inp("wk", [D, 4 * HD], BF16)
    wv = inp("wv", [D, 4 * HD], BF16)
    wo = inp("wo", [DCH, P, DCH, P], BF16)     # pre-tiled [o, p, a, f]
    wg = inp("wg", [FCH, P, DCH, P], F8)       # pre-tiled [fb, p, o, f], x16
    wu = inp("wu", [FCH, P, DCH, P], F8)       # pre-tiled [fb, p, o, f], x16
    wd = inp("wd", [4, DCH, P, DCH, P], F8)    # pre-tiled [sc, o, p, fs, f], x16
    bq = inp("bq", [P, H], F32)
    bk = inp("bk", [P, 4], F32)
    bvb = inp("bvb", [P, 4 * HD], F32)
    bo = inp("bo", [P, DCH], F32)
    bg = inp("bg", [P, FCH], F32)
    bu = inp("bu", [P, FCH], F32)
    bd = inp("bd", [P, DCH], F32)
    cosk = inp("cosk", [P, S], F32)
    sink = inp("sink", [P, S], F32)
    cosq = inp("cosq", [P, TOK], F32)
    sinq = inp("sinq", [P, TOK], F32)
    mask4 = inp("mask4", [P, 4, P], F32)   # per-core additive causal masks
    onesb = inp("onesb", [P, 1], BF16)
    epsv = inp("epsv", [P, 1], F32)
    out_t = nc.dram_tensor("out", [D, TOK], F32, kind="ExternalOutput").ap()

    xT_b3 = xT_b.rearrange("(o p) t -> p o t", p=P)
    xT_own3 = xT_own.rearrange("(o p) t -> p o t", p=P)

    with tile.TileContext(nc) as tc:
        with tc.tile_pool(name="consts", bufs=1) as consts, \
             tc.tile_pool(name="big", bufs=1) as big, \
             tc.tile_pool(name="dram", bufs=1, space="DRAM") as dram:
            onesb_s = consts.tile([P, 1], BF16)
            nc.sync.dma_start(onesb_s[:], onesb[:])
            eps_s = consts.tile([P, 1], F32)
            nc.sync.dma_start(eps_s[:], epsv[:])
            bq_s = consts.tile([P, H], F32)
            nc.sync.dma_start(bq_s[:], bq[:])
            bk_s = consts.tile([P, 4], F32)
            nc.sync.dma_start(bk_s[:], bk[:])
            bvb_s = consts.tile([P, 4 * HD], F32)
            nc.sync.dma_start(bvb_s[:], bvb[:])
            bo_s = consts.tile([P, DCH], F32)
            nc.sync.dma_start(bo_s[:], bo[:])
            bg_s = consts.tile([P, FCH], F32)
            nc.sync.dma_start(bg_s[:], bg[:])
            bu_s = consts.tile([P, FCH], F32)
            nc.sync.dma_start(bu_s[:], bu[:])
            bd_s = consts.tile([P, DCH], F32)
            nc.sync.dma_start(bd_s[:], bd[:])
            mask_s = consts.tile([P, 4, P], F32)
            nc.sync.dma_start(mask_s[:], mask4[:])
            cosq_s = consts.tile([P, TOK], F32)
            nc.sync.dma_start(cosq_s[:], cosq[:])
            sinq_s = consts.tile([P, TOK], F32)
            nc.sync.dma_start(sinq_s[:], sinq[:])

            # per-own-head K+V bounce (1 MB) and gather (4 MB) buffers
            kv_bounce = [dram.tile([2 * HDS], BF16, name=f"kvb{j}")
                         for j in range(4)]
            kv_gath = [dram.tile([4 * 2 * HDS], BF16, name=f"kvg{j}")
                       for j in range(4)]
            groups = [[0, 1, 2, 3], [4, 5, 6, 7]]

            # big SBUF slots (aliased across phases by tag)
            #   A (64K): nx_all (P1-P2) -> attention round K/V double-buffer
            #   B (32K): kv_ws (P2) -> acc (P5-P7)
            #   C (16K): cskt rope tables (P2) -> att_all (P4-P5) -> act2 (P6)
            #   D (16K): qt_all (P3-P4) -> nx2 (P6)
            nx_all = big.tile([P, DCH, S], BF16, tag="bigA", name="nx_all")
            kv_ws = big.tile([P, 2, DCH, 4 * HD], BF16, tag="bigB",
                             name="kv_ws")
            cskt = big.tile([P, 2, S], F32, tag="bigC", name="cskt")
            nc.sync.dma_start(cskt[:, 0, :], cosk[:])
            nc.sync.dma_start(cskt[:, 1, :], sink[:])
            nc.sync.dma_start(kv_ws[:, 0],
                              wk.rearrange("(o p) f -> p o f", p=P))
            nc.sync.dma_start(kv_ws[:, 1],
                              wv.rearrange("(o p) f -> p o f", p=P))

            def rope(pool, src, cos_t, sin_t, dst, tname):
                """src [128, n] f32 pre-rope -> dst [128, n] roped.

                cos_t/sin_t are [128, n] with the 64-row table duplicated in
                both partition halves. A half-swapped copy of src keeps every
                elementwise op lane-aligned:
                  ma = src*cos  -> [f*cos ; s*cos]
                  mb = swap(src)*sin -> [s*sin ; f*sin]
                  dst = [ma_top + mb_top ; mb_bot - ma_bot]
                """
                n = src.shape[-1]
                hh = HD // 2
                swp = pool.tile([P, n], F32, tag="rpsw", name=f"{tname}sw")
                nc.sync.dma_start(swp[0:hh, :], src[hh:P, :])
                nc.sync.dma_start(swp[hh:P, :], src[0:hh, :])
                ma = pool.tile([P, n], F32, tag="rp1", name=f"{tname}ma")
                mb = pool.tile([P, n], F32, tag="rp2", name=f"{tname}mb")
                nc.vector.tensor_mul(out=ma[:], in0=src[:], in1=cos_t)
                nc.vector.tensor_mul(out=mb[:], in0=swp[:], in1=sin_t)
                nc.vector.tensor_add(out=dst[0:hh], in0=ma[0:hh],
                                     in1=mb[0:hh])
                nc.vector.tensor_sub(out=dst[hh:P], in0=mb[hh:P],
                                     in1=ma[hh:P])

            # ---- P1+P2a fused: per tb, rmsnorm into nx_all then V proj ----
            # (the V matmuls of tb keep the PE busy while the scalar/vector
            # engines normalize tb+1)
            with tc.tile_pool(name="p1", bufs=3) as pool, \
                 tc.tile_pool(name="p1ps", bufs=2, space="PSUM") as psum, \
                 tc.tile_pool(name="p2aps", bufs=1, space="PSUM") as psumv:
                for tb in range(S // TOK):
                    cols = bass.ts(tb, TOK)
                    sumsq = psum.tile([1, TOK], F32, tag="n1ss",
                                      name=f"n1ss{tb}")
                    for o in range(DCH):
                        xc = pool.tile([P, TOK], F32R, tag="n1x",
                                       name=f"n1x{tb}_{o}")
                        nc.sync.dma_start(xc[:], xT_b3[:, o, cols])
                        sq = pool.tile([P, TOK], BF16, tag="n1sq",
                                       name=f"n1sq{tb}_{o}")
                        nc.scalar.activation(sq[:], xc[:], AFT.Square)
                        nc.tensor.matmul(sumsq[:], lhsT=onesb_s[:], rhs=sq[:],
                                         start=(o == 0), stop=(o == DCH - 1))
                    rms = pool.tile([1, TOK], F32, tag="n1rms",
                                    name=f"n1rms{tb}")
                    nc.scalar.activation(rms[:], sumsq[:], AFT.Sqrt,
                                         scale=1.0 / D, bias=eps_s[:1])
                    rec = pool.tile([1, TOK], F32, tag="n1rec",
                                    name=f"n1rec{tb}")
                    nc.vector.reciprocal(rec[:], rms[:])
                    rbc = pool.tile([P, TOK], F32, tag="n1rbc",
                                    name=f"n1rbc{tb}")
                    nc.gpsimd.partition_broadcast(rbc[:], rec[:])
                    for o in range(DCH):
                        xc2 = pool.tile([P, TOK], F32R, tag="n1x2",
                                        name=f"n1x2{tb}_{o}")
                        nc.sync.dma_start(xc2[:], xT_b3[:, o, cols])
                        nc.vector.tensor_mul(out=nx_all[:, o, cols],
                                             in0=xc2[:].bitcast(F32),
                                             in1=rbc[:])
                    # V proj for this tb
                    vps = [psumv.tile([P, 4 * HD], F32, tag=f"vps{i}",
                                      name=f"vps{i}_{tb}") for i in range(4)]
                    for o in range(DCH):
                        st, sp = (o == 0), (o == DCH - 1)
                        for ts_ in range(4):
                            nc.tensor.matmul(
                                vps[ts_][:],
                                lhsT=nx_all[:, o,
                                            bass.ds(tb * TOK + ts_ * P, P)],
                                rhs=kv_ws[:, 1, o, :], start=st, stop=sp)
                    for ts_ in range(4):
                        vsb = pool.tile([P, 4 * HD], BF16, tag="vsb",
                                        name=f"vsb{tb}_{ts_}")
                        nc.vector.tensor_add(out=vsb[:], in0=vps[ts_][:],
                                             in1=bvb_s[:])
                        row0 = tb * TOK + ts_ * P
                        for j in range(4):
                            vdst = kv_bounce[j][HDS:2 * HDS].rearrange(
                                "(s hd) -> s hd", hd=HD)
                            nc.sync.dma_start(
                                vdst[bass.ds(row0, P), :],
                                vsb[:, bass.ts(j, HD)])

            # ---- P2b: K proj per own head j + rope -> AG_j (K+V) ----
            with tc.tile_pool(name="p2b", bufs=2) as pool, \
                 tc.tile_pool(name="p2bps", bufs=2, space="PSUM") as psum:
                for j in range(4):
                    kdst = kv_bounce[j][0:HDS].rearrange("(hd s) -> hd s", s=S)
                    for tb in range(S // TOK):
                        cols = bass.ts(tb, TOK)
                        kps = psum.tile([P, TOK], F32, tag="kps",
                                        name=f"kps{j}_{tb}")
                        for o in range(DCH):
                            nc.tensor.matmul(
                                kps[:], lhsT=kv_ws[:, 0, o, bass.ts(j, HD)],
                                rhs=nx_all[:, o, cols],
                                start=(o == 0), stop=(o == DCH - 1))
                        kb_t = pool.tile([P, TOK], F32, tag="kbias",
                                         name=f"kbias{j}_{tb}")
                        nc.scalar.activation(kb_t[:], kps[:], AFT.Identity,
                                             bias=bk_s[:, j:j + 1])
                        krt = pool.tile([P, TOK], BF16, tag="kroped",
                                        name=f"kroped{j}_{tb}")
                        rope(pool, kb_t[:], cskt[:, 0, cols], cskt[:, 1, cols],
                             krt[:], f"kr{j}_{tb}")
                        nc.sync.dma_start(kdst[:, cols], krt[:])
                    nc.gpsimd.collective_compute(
                        "AllGather", mybir.AluOpType.bypass,
                        ins=[kv_bounce[j][:].opt()],
                        outs=[kv_gath[j][:].opt()],
                        replica_groups=groups)

            # ---- P3: Q projections for own tokens, all heads + rope ----
            qt_all = big.tile([P, H, TOK], BF16, tag="bigD", name="qt_all")
            with tc.tile_pool(name="p3", bufs=2) as pool, \
                 tc.tile_pool(name="p3q", bufs=1) as poolq, \
                 tc.tile_pool(name="p3ps", bufs=2, space="PSUM") as psum:
                nxq = poolq.tile([P, DCH, TOK], BF16, tag="nxq")
                rbcq = poolq.tile([P, TOK], F32, tag="rbcq")
                sumsq = psum.tile([1, TOK], F32, tag="qss")
                for o in range(DCH):
                    xc = pool.tile([P, TOK], F32R, tag="qx", name=f"qx{o}")
                    nc.sync.dma_start(xc[:], xT_own3[:, o, :])
                    sq = pool.tile([P, TOK], BF16, tag="qsq", name=f"qsq{o}")
                    nc.scalar.activation(sq[:], xc[:], AFT.Square)
                    nc.tensor.matmul(sumsq[:], lhsT=onesb_s[:], rhs=sq[:],
                                     start=(o == 0), stop=(o == DCH - 1))
                rms = pool.tile([1, TOK], F32, tag="qrms")
                nc.scalar.activation(rms[:], sumsq[:], AFT.Sqrt,
                                     scale=1.0 / D, bias=eps_s[:1])
                rec = pool.tile([1, TOK], F32, tag="qrec")
                nc.vector.reciprocal(rec[:], rms[:])
                nc.gpsimd.partition_broadcast(rbcq[:], rec[:])
                for o in range(DCH):
                    xc2 = pool.tile([P, TOK], F32R, tag="qx2", name=f"qx2{o}")
                    nc.sync.dma_start(xc2[:], xT_own3[:, o, :])
                    nc.vector.tensor_mul(out=nxq[:, o, :],
                                         in0=xc2[:].bitcast(F32),
                                         in1=rbcq[:])
                for h in range(H):
                    wq_s = pool.tile([P, DCH, HD], BF16, tag="wqs",
                                     name=f"wqs{h}")
                    nc.sync.dma_start(wq_s[:], wq[h])
                    qp = psum.tile([P, TOK], F32, tag="qps", name=f"qps{h}")
                    for o in range(DCH):
                        nc.tensor.matmul(qp[:], lhsT=wq_s[:, o, :],
                                         rhs=nxq[:, o, :],
                                         start=(o == 0),
                                         stop=(o == DCH - 1))
                    qb_t = pool.tile([P, TOK], F32, tag="qbias",
                                     name=f"qbias{h}")
                    nc.scalar.activation(qb_t[:], qp[:], AFT.Identity,
                                         bias=bq_s[:, h:h + 1])
                    rope(pool, qb_t[:], cosq_s[:], sinq_s[:],
                         qt_all[:, h, :], f"qr{h}")

            # ---- P4: attention rounds (causal, stride-4 query chunks) ----
            # round j processes heads {4m+j}; K/V from AG_j.  At key block kb
            # only query slots i >= kb//4 are live (suffix); the first live
            # slot gets the additive mask mask_s[:, kb%4, :].
            att_all = big.tile([P, H, TOK], BF16, tag="bigC", name="att_all")
            kvr = big.tile([P, 2, 4, 2, S], BF16, tag="bigA", name="kvr")
            with tc.tile_pool(name="p4", bufs=3) as pool, \
                 tc.tile_pool(name="p4ps", bufs=2, space="PSUM") as psum:
                for j in range(4):
                    r2 = j % 2
                    for m in range(4):
                        h = 4 * m + j
                        ktg = kvr[:, r2, m, 0, :]
                        nc.sync.dma_start(
                            ktg,
                            kv_gath[j][bass.ds(m * 2 * HDS, HDS)].rearrange(
                                "(hd s) -> hd s", s=S))
                        vg = kvr[:, r2, m, 1, :].rearrange(
                            "p (kb hd) -> p kb hd", hd=HD)
                        nc.sync.dma_start(
                            vg,
                            kv_gath[j][bass.ds(m * 2 * HDS + HDS,
                                               HDS)].rearrange(
                                "(kb p hd) -> p kb hd", p=P, hd=HD))
                        den = psum.tile([1, TOK], F32, tag="denps",
                                        name=f"den{h}")
                        op = psum.tile([P, TOK], F32, tag="outps",
                                       name=f"ops{h}")
                        for kb in range(KB):
                            off = (kb // 4) * P
                            stp = psum.tile([P, TOK], F32, tag="stps",
                                            name=f"st{h}_{kb}")
                            nc.tensor.matmul(stp[:, off:TOK],
                                             lhsT=ktg[:, bass.ts(kb, P)],
                                             rhs=qt_all[:, h, off:TOK],
                                             start=True, stop=True)
                            nc.vector.tensor_add(out=stp[:, off:off + P],
                                                 in0=stp[:, off:off + P],
                                                 in1=mask_s[:, kb % 4, :])
                            est = pool.tile([P, TOK], BF16, tag="est",
                                            name=f"est{h}_{kb}")
                            nc.scalar.activation(est[:, off:TOK],
                                                 stp[:, off:TOK],
                                                 AFT.Exp, scale=SCALE)
                            st, sp = (kb == 0), (kb == KB - 1)
                            nc.tensor.matmul(den[:, off:TOK], lhsT=onesb_s[:],
                                             rhs=est[:, off:TOK],
                                             start=st, stop=sp,
                                             skip_group_check=True)
                            nc.tensor.matmul(op[:, off:TOK],
                                             lhsT=vg[:, kb, :],
                                             rhs=est[:, off:TOK],
                                             start=st, stop=sp,
                                             skip_group_check=True)
                        recd = pool.tile([1, TOK], F32, tag="recd",
                                         name=f"recd{h}")
                        nc.vector.reciprocal(recd[:], den[:])
                        rdb = pool.tile([P, TOK], F32, tag="rdb",
                                        name=f"rdb{h}")
                        nc.gpsimd.partition_broadcast(rdb[:], recd[:])
                        nc.vector.tensor_mul(out=att_all[:, h, :], in0=op[:],
                                             in1=rdb[:])

            # ---- P5: WO for own tokens + residual -> acc (= x2T) ----
            acc = big.tile([P, DCH, TOK], F32, tag="bigB", name="acc")
            with tc.tile_pool(name="p5", bufs=2) as pool, \
                 tc.tile_pool(name="p5ps", bufs=2, space="PSUM") as psum:
                for o in range(DCH):
                    wo_s = pool.tile([P, DCH, P], BF16, tag="wos",
                                     name=f"wos{o}")
                    nc.sync.dma_start(wo_s[:], wo[o])
                    x2p = psum.tile([P, TOK], F32, tag="x2ps", name=f"x2ps{o}")
                    for h in range(H):
                        nc.tensor.matmul(x2p[:], lhsT=wo_s[:, h, :],
                                         rhs=att_all[:, h, :],
                                         start=(h == 0), stop=(h == H - 1))
                    x2pre = pool.tile([P, TOK], F32, tag="x2pre",
                                      name=f"x2pre{o}")
                    nc.scalar.activation(x2pre[:], x2p[:], AFT.Identity,
                                         bias=bo_s[:, o:o + 1])
                    xres = pool.tile([P, TOK], F32R, tag="xres",
                                     name=f"xres{o}")
                    nc.sync.dma_start(xres[:], xT_own3[:, o, :])
                    nc.vector.tensor_add(out=acc[:, o, :], in0=x2pre[:],
                                         in1=xres[:].bitcast(F32))

            # ---- P6: norm2 + FFN (fp8 weights/activations, DoubleRow) ----
            nx2 = big.tile([P, DCH, TOK], F8, tag="bigD", name="nx2")
            act2 = big.tile([P, 2, DCH, TOK], F8, tag="bigC", name="act2")
            with tc.tile_pool(name="p6w", bufs=3) as wpool6, \
                 tc.tile_pool(name="p6", bufs=2) as pool, \
                 tc.tile_pool(name="p6ps", bufs=2, space="PSUM") as psum:
                rbc2 = pool.tile([P, TOK], F32, tag="rbc2")
                sumsq = psum.tile([1, TOK], F32, tag="n2ss")
                for o in range(DCH):
                    sq = pool.tile([P, TOK], BF16, tag="n2sq", name=f"n2sq{o}")
                    nc.scalar.activation(sq[:], acc[:, o, :], AFT.Square)
                    nc.tensor.matmul(sumsq[:], lhsT=onesb_s[:], rhs=sq[:],
                                     start=(o == 0), stop=(o == DCH - 1))
                rms = pool.tile([1, TOK], F32, tag="n2rms")
                nc.scalar.activation(rms[:], sumsq[:], AFT.Sqrt,
                                     scale=1.0 / D, bias=eps_s[:1])
                rec = pool.tile([1, TOK], F32, tag="n2rec")
                nc.vector.reciprocal(rec[:], rms[:])
                nc.gpsimd.partition_broadcast(rbc2[:], rec[:])
                for o in range(DCH):
                    nc.vector.tensor_mul(out=nx2[:, o, :], in0=acc[:, o, :],
                                         in1=rbc2[:])
                # fold b_down into acc now (added once)
                for o in range(DCH):
                    nc.vector.tensor_scalar_add(acc[:, o, :], acc[:, o, :],
                                                bd_s[:, o:o + 1])
                for sc in range(4):
                    for fs in range(DCH):
                        f = sc * DCH + fs
                        wg_s = wpool6.tile([P, DCH, P], F8, tag="wgs",
                                           name=f"wgs{f}")
                        nc.sync.dma_start(wg_s[:], wg[f])
                        wu_s = wpool6.tile([P, DCH, P], F8, tag="wus",
                                           name=f"wus{f}")
                        nc.sync.dma_start(wu_s[:], wu[f])
                        gp = psum.tile([P, TOK], F32, tag="gps", name=f"gps{f}")
                        up = psum.tile([P, TOK], F32, tag="ups", name=f"ups{f}")
                        for oj in range(DCH // 2):
                            st, sp = (oj == 0), (oj == DCH // 2 - 1)
                            o2 = bass.ds(2 * oj, 2)
                            nc.tensor.matmul(gp[:], lhsT=wg_s[:, o2, :],
                                             rhs=nx2[:, o2, :], start=st,
                                             stop=sp, perf_mode=DR)
                            nc.tensor.matmul(up[:], lhsT=wu_s[:, o2, :],
                                             rhs=nx2[:, o2, :], start=st,
                                             stop=sp, perf_mode=DR)
                        gs = pool.tile([P, TOK], F32, tag="gsig", name=f"gs{f}")
                        nc.scalar.activation(gs[:], gp[:], AFT.Silu,
                                             scale=1.0 / W8SCALE,
                                             bias=bg_s[:, f:f + 1])
                        us = pool.tile([P, TOK], F32, tag="usig", name=f"us{f}")
                        nc.scalar.activation(us[:], up[:], AFT.Identity,
                                             scale=1.0 / W8SCALE,
                                             bias=bu_s[:, f:f + 1])
                        nc.vector.tensor_mul(out=act2[:, sc % 2, fs, :],
                                             in0=gs[:], in1=us[:])
                    for o in range(DCH):
                        wd_s = wpool6.tile([P, DCH, P], F8, tag="wds",
                                           name=f"wds{sc}_{o}")
                        nc.sync.dma_start(wd_s[:], wd[sc, o])
                        dp = psum.tile([P, TOK], F32, tag="dps",
                                       name=f"dps{sc}_{o}")
                        for fj in range(DCH // 2):
                            f2 = bass.ds(2 * fj, 2)
                            nc.tensor.matmul(dp[:], lhsT=wd_s[:, f2, :],
                                             rhs=act2[:, sc % 2, f2, :],
                                             start=(fj == 0),
                                             stop=(fj == DCH // 2 - 1),
                                             perf_mode=DR)
                        nc.vector.scalar_tensor_tensor(
                            out=acc[:, o, :], in0=dp[:],
                            scalar=1.0 / W8SCALE, in1=acc[:, o, :],
                            op0=ALU.mult, op1=ALU.add)

            # ---- P7: write transposed output ----
            nc.sync.dma_start(
                out_t.rearrange("(o p) t -> p o t", p=P), acc[:])

    nc.compile()
    return nc


def _prepare_inputs(inputs):
    """Build the 8 per-core in_maps from the full problem inputs."""
    x = np.ascontiguousarray(inputs["x"], dtype=np.float32)   # [B, S, D]
    n1 = np.asarray(inputs["norm1_w"], dtype=np.float32)
    n2 = np.asarray(inputs["norm2_w"], dtype=np.float32)
    wq_f = np.ascontiguousarray(n1[:, None] * np.asarray(inputs["wq"], np.float32))
    wk_f = n1[:, None] * np.asarray(inputs["wk"], np.float32)
    wv_f = n1[:, None] * np.asarray(inputs["wv"], np.float32)
    wo_f = np.ascontiguousarray(np.asarray(inputs["wo"], np.float32))
    wg_f = np.ascontiguousarray(n2[:, None] * np.asarray(inputs["w_gate"], np.float32))
    wu_f = np.ascontiguousarray(n2[:, None] * np.asarray(inputs["w_up"], np.float32))
    wd_f = np.ascontiguousarray(np.asarray(inputs["w_down"], np.float32))
    bq = np.asarray(inputs["bq"], np.float32).reshape(H, P).T.copy()
    bo = np.asarray(inputs["bo"], np.float32).reshape(DCH, P).T.copy()
    bg = np.asarray(inputs["b_gate"], np.float32).reshape(FCH, P).T.copy()
    bu = np.asarray(inputs["b_up"], np.float32).reshape(FCH, P).T.copy()
    bd = np.asarray(inputs["b_down"], np.float32).reshape(DCH, P).T.copy()
    bk_full = np.asarray(inputs["bk"], np.float32)
    bv_full = np.asarray(inputs["bv"], np.float32)

    cosk, sink = _rope_tables(np.arange(S))
    onesb_np = np.ones((P, 1), ml_dtypes.bfloat16)
    epsv = np.full((P, 1), EPS, np.float32)

    xT = [np.ascontiguousarray(x[b].T) for b in range(B)]      # [D, S]
    bf = ml_dtypes.bfloat16
    # pre-tiled layouts so every weight-tile DMA is one contiguous block
    wq_b = np.ascontiguousarray(
        wq_f.astype(bf).reshape(DCH, P, H, HD).transpose(2, 1, 0, 3))
    wk_b = wk_f.astype(bf)
    wv_b = wv_f.astype(bf)
    wo_b = np.ascontiguousarray(
        wo_f.astype(bf).reshape(DCH, P, DCH, P).transpose(2, 1, 0, 3))
    f8 = ml_dtypes.float8_e4m3
    wg_b = np.ascontiguousarray(
        (W8SCALE * wg_f).astype(f8).reshape(DCH, P, FCH, P).transpose(2, 1, 0, 3))
    wu_b = np.ascontiguousarray(
        (W8SCALE * wu_f).astype(f8).reshape(DCH, P, FCH, P).transpose(2, 1, 0, 3))
    wd_b = np.ascontiguousarray(
        (W8SCALE * wd_f).astype(f8).reshape(4, DCH, P, DCH, P).transpose(0, 3, 2, 1, 4))

    tri = np.where(np.arange(P)[:, None] > np.arange(P)[None, :],
                   NEG, 0.0).astype(np.float32)
    in_maps = []
    for c in range(NC):
        b, p = c // 4, c % 4
        kv0 = 4 * p * HD                                        # head-col base
        # owned query chunks: slots i = 0..3 hold chunk p + 4i (128 tokens)
        qpos = np.concatenate([np.arange(P) + (p + 4 * i) * P
                               for i in range(4)])
        cosq, sinq = _rope_tables(qpos)
        mask4 = np.empty((P, 4, P), np.float32)
        for j in range(4):
            if j < p:
                mask4[:, j, :] = 0.0
            elif j == p:
                mask4[:, j, :] = tri
            else:
                mask4[:, j, :] = NEG
        in_maps.append({
            "xT_b": xT[b],
            "xT_own": np.ascontiguousarray(xT[b][:, qpos]),
            "wq": wq_b,
            "wk": np.ascontiguousarray(wk_b[:, kv0:kv0 + 4 * HD]),
            "wv": np.ascontiguousarray(wv_b[:, kv0:kv0 + 4 * HD]),
            "wo": wo_b,
            "wg": wg_b, "wu": wu_b, "wd": wd_b,
            "bq": bq,
            "bk": bk_full[kv0:kv0 + 4 * HD].reshape(4, P).T.copy(),
            "bvb": np.tile(bv_full[kv0:kv0 + 4 * HD][None, :], (P, 1)).copy(),
            "bo": bo, "bg": bg, "bu": bu, "bd": bd,
            "cosk": cosk, "sink": sink, "cosq": cosq, "sinq": sinq,
            "mask4": mask4, "onesb": onesb_np, "epsv": epsv,
        })
    return in_maps


def kernel(**inputs):
    global LAST_RESULT
    if "nc" not in _CACHE:
        _CACHE["nc"] = _build_program()
    nc = _CACHE["nc"]
    in_maps = _prepare_inputs(inputs)
    trace = bool(int(os.environ.get("BASS_TRACE", "0")))
    res = run_bass_kernel_spmd(nc, in_maps, core_ids=list(range(NC)),
                               trace=trace)
    LAST_RESULT = res
    # assemble: core c owns query chunks {p+4i} of batch c//4 (p = c%4)
    full = np.empty((B * S, D), np.float32)
    for c in range(NC):
        b, p = c // 4, c % 4
        oc = res.results[c]["out"]                      # [D, TOK]
        for i in range(4):
            qc = p + 4 * i
            full[b * S + qc * P:b * S + (qc + 1) * P, :] = \
                oc[:, i * P:(i + 1) * P].T
    return full.reshape(B, S, D)


if __name__ == "__main__":
    print("import as module; use kernel(**inputs)")
